# revision 1
# baseline (speedup 1.0000x reference)
"""Contrastive loss (supervised NT-Xent style) on 8 Trainium2 NeuronCores.

Reference computation (N=8192, D=256, C=64 classes, T=0.5):
    sim   = (E @ E.T) / T
    max_i = row max of sim           (== sim_ii because rows are unit-norm)
    den_i = sum_{j != i} exp(sim_ij - max_i)
    loss  = mean over positive pairs (label match, i != j) of
            (log den_i + max_i - sim_ij)

Key algebraic restructuring: the positive-pair sim sum only enters the loss
globally, and
    sum_{i != j, lab_i == lab_j} sim_ij = (sum_c ||G_c||^2 - sum_i ||e_i||^2)/T
with G_c = sum of embeddings in class c.  So no per-pair masking is needed on
device; each core produces
    - den_full_i  (exp row sums, diagonal included -> host subtracts 1)
    - sumsq_i     (||e_i||^2, gives max_i = 2*sumsq_i)
    - g_part[c,d] (class sums over the core's 1024 rows)
and the host combines them with label bincounts into the scalar loss.

Sharding: rows split across 8 cores; each core computes its [1024, 8192] sim
block against the full embedding set (bf16 matmul, fp32 PSUM), with the exp
row-sum fused into the ScalarEngine activation pass via accum_out.
"""

import numpy as np
import ml_dtypes

import concourse.bass as bass
import concourse.bacc as bacc
import concourse.mybir as mybir
import concourse.tile as tile
from concourse.bass_utils import run_bass_kernel_spmd

N = 8192
D = 256
C = 64
TEMP = 0.5
N_CORES = 8
M = N // N_CORES          # 1024 rows per core
P = 128                   # partitions
MT = M // P               # 8 m-tiles per core
CHUNK = 512               # fp32 moving-operand / PSUM-bank width
QW = 2048                 # psum ping-pong tile width (4 banks)
NQ = N // QW              # 4 quarters per m-tile row

_F32 = mybir.dt.float32
_BF16 = mybir.dt.bfloat16
_BF16_NP = ml_dtypes.bfloat16


def build_nc(enable_asserts: bool = False):
    nc = bacc.Bacc(
        "TRN2",
        target_bir_lowering=False,
        debug=False,
        enable_asserts=enable_asserts,
        num_devices=N_CORES,
    )

    # chunk-major layout: [k, s, p, c] so each [128, 512] chunk is contiguous
    embT = nc.dram_tensor("embT", [2, N // CHUNK, P, CHUNK], _BF16, kind="ExternalInput").ap()
    embT_rows = nc.dram_tensor("embT_rows", [D, M], _BF16, kind="ExternalInput").ap()
    emb_rows = nc.dram_tensor("emb_rows", [M, D], _BF16, kind="ExternalInput").ap()
    onehot_rows = nc.dram_tensor("onehot_rows", [M, C], _BF16, kind="ExternalInput").ap()

    # row_stats[:, 0:8]  = den_full per m-tile,  row_stats[:, 8:16] = sumsq
    row_stats_d = nc.dram_tensor("row_stats", [P, 2 * MT], _F32, kind="ExternalOutput").ap()
    g_part_d = nc.dram_tensor("g_part", [C, D], _F32, kind="ExternalOutput").ap()

    with tile.TileContext(nc) as tc:
        with (
            tc.tile_pool(name="big", bufs=1) as big,
            tc.tile_pool(name="small", bufs=1) as small,
            tc.tile_pool(name="psum", bufs=2, space=bass.MemorySpace.PSUM) as psum,
        ):
            # ---- persistent SBUF residents ----
            embT_sb = [big.tile([P, N], _BF16, tag=f"embT{k}", name=f"embT_sb{k}") for k in range(2)]
            embTr_sb = [big.tile([P, M], _BF16, tag=f"embTr{k}", name=f"embTr_sb{k}") for k in range(2)]
            embr_sb = big.tile([P, MT * D], _BF16, tag="embr")      # natural rows
            oh_sb = big.tile([P, MT * C], _BF16, tag="oh")          # onehot rows

            # cols 0:32 = per-(m,q) partials; cols 32:34 = first-half partials
            # of the split (q0, m<2) tiles, folded in before the final reduce
            denom_parts = small.tile([P, MT * NQ + 2], _F32, tag="dparts")
            row_stats = small.tile([P, 2 * MT], _F32, tag="rstats")
            negmax = small.tile([P, MT], _F32, tag="negmax")
            sq_junk = small.tile([P, D], _F32, tag="sqjunk")
            g_sb = small.tile([C, D], _F32, tag="gsb")
            dummy = small.tile([P, 1], _F32, tag="dummy")
            warm = small.tile([P, P], _BF16, tag="warm")

            # ---- t=0: hoist the ACT exp table load; warm the PE HAM ----
            nc.gpsimd.memset(dummy[:], 0.0)
            nc.scalar.activation(
                out=dummy[:], in_=dummy[:],
                func=mybir.ActivationFunctionType.Exp, bias=0.0, scale=1.0,
            )
            nc.gpsimd.memset(warm[:], 0.0)
            warm_ps = psum.tile([P, P], _F32, tag="ps", name="warm_ps")
            for _ in range(24):
                nc.tensor.matmul(warm_ps[:], lhsT=warm[:], rhs=warm[:], start=True, stop=True)

            # ---- input DMAs (issue order == priority order) ----
            # lhsT first (first matmuls need it), then embT in consumption
            # order; emb_rows m0 early for the first negmax. q2/q3 stream on
            # the gpsimd SWDGE queue in parallel with the sync HWDGE queue.
            # Split the pre-first-EXP stream across both HWDGE queues:
            # sync: lhsT + k0 of q0/q1;  scalar (idle until first EXP): k1 of
            # q0/q1.  Everything later goes on sync.
            nc.sync.dma_start(out=embTr_sb[0][:], in_=embT_rows[0:P, :])
            nc.scalar.dma_start(out=embTr_sb[1][:], in_=embT_rows[P:2 * P, :])
            for s in range(QW // CHUNK):
                nc.sync.dma_start(
                    out=embT_sb[0][:, s * CHUNK:(s + 1) * CHUNK],
                    in_=embT[0, s],
                )
                nc.scalar.dma_start(
                    out=embT_sb[1][:, s * CHUNK:(s + 1) * CHUNK],
                    in_=embT[1, s],
                )
            nc.sync.dma_start(out=embr_sb[:, 0:D], in_=emb_rows[0:P, :])
            nc.scalar.dma_start(
                out=embT_sb[1][:, QW:2 * QW].rearrange("p (s c) -> p s c", c=CHUNK),
                in_=embT[1, QW // CHUNK:2 * QW // CHUNK].rearrange("s p c -> p s c"),
            )
            nc.sync.dma_start(
                out=embr_sb[:, D:].rearrange("p (m d) -> p m d", d=D),
                in_=emb_rows[P:, :].rearrange("(m p) d -> p m d", p=P),
            )
            nc.sync.dma_start(
                out=embT_sb[0][:, QW:2 * QW].rearrange("p (s c) -> p s c", c=CHUNK),
                in_=embT[0, QW // CHUNK:2 * QW // CHUNK].rearrange("s p c -> p s c"),
            )
            for q in range(2, NQ):
                for k in range(2):
                    nc.sync.dma_start(
                        out=embT_sb[k][:, q * QW:(q + 1) * QW].rearrange("p (s c) -> p s c", c=CHUNK),
                        in_=embT[k, q * QW // CHUNK:(q + 1) * QW // CHUNK].rearrange("s p c -> p s c"),
                    )
            nc.sync.dma_start(
                out=oh_sb[:].rearrange("p (m c) -> p m c", c=C),
                in_=onehot_rows[:].rearrange("(m p) c -> p m c", p=P),
            )

            # ---- per-row sumsq (-> max_i = 2*sumsq_i) ----
            # (tensor_tensor_reduce crashes TRN2 here; use mul + reduce)
            for m in range(MT):
                nc.vector.tensor_mul(
                    sq_junk[:],
                    embr_sb[:, m * D:(m + 1) * D],
                    embr_sb[:, m * D:(m + 1) * D],
                )
                nc.vector.tensor_reduce(
                    out=row_stats[:, MT + m:MT + m + 1],
                    in_=sq_junk[:],
                    axis=mybir.AxisListType.X,
                    op=mybir.AluOpType.add,
                )
                # per-m so the first ACT op doesn't wait on all 8 sumsq
                nc.vector.tensor_scalar_mul(
                    out=negmax[:, m:m + 1],
                    in0=row_stats[:, MT + m:MT + m + 1],
                    scalar1=-2.0,
                )

            # ---- main loop: sim chunks + fused exp row-sum ----
            # q outer / m inner: all 8 m-tiles consume quarter q while the
            # DMA stream for quarters q+1.. runs behind the compute.
            for q in range(NQ):
                for m in range(MT):
                    ps = psum.tile([P, QW], _F32, tag="ps")
                    for k in range(2):
                        for c4 in range(QW // CHUNK):
                            col = q * QW + c4 * CHUNK
                            nc.tensor.matmul(
                                ps[:, c4 * CHUNK:(c4 + 1) * CHUNK],
                                lhsT=embTr_sb[k][:, m * P:(m + 1) * P],
                                rhs=embT_sb[k][:, col:col + CHUNK],
                                start=(k == 0),
                                stop=(k == 1),
                            )
                    nc.scalar.activation(
                        out=ps[:],
                        in_=ps[:],
                        func=mybir.ActivationFunctionType.Exp,
                        bias=negmax[:, m:m + 1],
                        scale=2.0,
                        accum_out=denom_parts[:, m * NQ + q:m * NQ + q + 1],
                    )

            # ---- class sums over this core's rows: g[c, d] ----
            # (after the main loop: lowest priority, fills PE idle slack)
            g_ps = psum.tile([C, D], _F32, tag="ps")
            for j in range(MT):
                nc.tensor.matmul(
                    g_ps[:],
                    lhsT=oh_sb[:, j * C:(j + 1) * C],
                    rhs=embr_sb[:, j * D:(j + 1) * D],
                    start=(j == 0),
                    stop=(j == MT - 1),
                )
            nc.vector.tensor_copy(g_sb[:], g_ps[:])
            nc.sync.dma_start(out=g_part_d[:], in_=g_sb[:])

            # ---- fold quarter partials -> den_full per m-tile ----
            # fold the split-tile first-half partials into the q0 slots
            for m in range(2):
                nc.vector.tensor_add(
                    denom_parts[:, m * NQ:m * NQ + 1],
                    denom_parts[:, m * NQ:m * NQ + 1],
                    denom_parts[:, MT * NQ + m:MT * NQ + m + 1],
                )
            # per-m so only the last reduce waits on the final EXP
            for m in range(MT):
                nc.vector.tensor_reduce(
                    out=row_stats[:, m:m + 1],
                    in_=denom_parts[:, m * NQ:(m + 1) * NQ],
                    axis=mybir.AxisListType.X,
                    op=mybir.AluOpType.add,
                )
            nc.sync.dma_start(out=row_stats_d[:], in_=row_stats[:])

    nc.compile()
    return nc


_NC_CACHE = None


def _get_nc():
    global _NC_CACHE
    if _NC_CACHE is None:
        _NC_CACHE = build_nc()
    return _NC_CACHE


def make_in_maps(embeddings: np.ndarray, labels: np.ndarray):
    emb = np.asarray(embeddings, dtype=np.float32)
    labels = np.asarray(labels).astype(np.int64)
    emb16 = emb.astype(_BF16_NP)
    embT16 = np.ascontiguousarray(emb16.T)
    # chunk-major: [k, s, p, c] with each [128, 512] chunk contiguous
    embT_t = np.ascontiguousarray(
        embT16.reshape(2, P, N // CHUNK, CHUNK).transpose(0, 2, 1, 3)
    )
    onehot = (labels[:, None] == np.arange(C)[None, :]).astype(_BF16_NP)

    in_maps = []
    for c in range(N_CORES):
        r0, r1 = c * M, (c + 1) * M
        in_maps.append(
            {
                "embT": embT_t,
                "embT_rows": np.ascontiguousarray(embT16[:, r0:r1]),
                "emb_rows": np.ascontiguousarray(emb16[r0:r1, :]),
                "onehot_rows": np.ascontiguousarray(onehot[r0:r1, :]),
            }
        )
    return in_maps


def finalize(results, labels: np.ndarray) -> np.float32:
    labels = np.asarray(labels).astype(np.int64)
    den_full = np.empty(N, dtype=np.float64)
    sumsq = np.empty(N, dtype=np.float64)
    G = np.zeros((C, D), dtype=np.float64)
    for c in range(N_CORES):
        rs = np.asarray(results[c]["row_stats"], dtype=np.float64)  # [P, 2*MT]
        for m in range(MT):
            base = c * M + m * P
            den_full[base:base + P] = rs[:, m]
            sumsq[base:base + P] = rs[:, MT + m]
        G += np.asarray(results[c]["g_part"], dtype=np.float64)

    counts = np.bincount(labels, minlength=C)
    npos = counts[labels] - 1.0
    n_pos = npos.sum()

    max_i = 2.0 * sumsq
    den = den_full - 1.0            # drop the diagonal exp(0) term
    logden = np.log(den)
    pos_sim_total = 2.0 * ((G * G).sum() - sumsq.sum())  # (1/T) * (...)
    numer = (npos * (logden + max_i)).sum() - pos_sim_total
    return np.float32(numer / n_pos)


def _run(inputs, trace: bool = False, **kwargs):
    nc = _get_nc()
    in_maps = make_in_maps(inputs["embeddings"], inputs["epitope_labels"])
    return run_bass_kernel_spmd(nc, in_maps, list(range(N_CORES)), trace=trace, **kwargs)


def kernel(embeddings, epitope_labels) -> np.ndarray:
    res = _run({"embeddings": embeddings, "epitope_labels": epitope_labels})
    return finalize(res.results, epitope_labels)



# revision 10
# speedup vs baseline: 1.0787x; 1.0787x over previous
"""Contrastive loss (supervised NT-Xent style) on 8 Trainium2 NeuronCores.

Reference (N=8192, D=256, C=64, T=0.5):
    sim   = (E @ E.T) / T = 2*(e_i . e_j)
    loss  = mean over positive pairs (label match, i != j) of
            (log sum_{j != i} exp(sim_ij) - sim_ij)
(The reference's row-max shift cancels exactly: log-sum-exp + max is
shift-invariant, and |sim| <= 2 so no overflow protection is needed.)

Device work (the N^2 part): den_full_i = sum_j exp(2 e_i.e_j), rows
sharded across 8 cores (1024 rows/core), each against the full 8192
columns. Everything O(N*D) — class sums G_c, ||e_i||^2, the diagonal
correction, bincounts, logs — runs on host in float64.

Per-core engine split (the exp over 8192x8192 is the roofline):
  - PE:   fp8(e4m3) DoubleRow matmuls (K=256 in one pass, 2 fp8
          weights/cell) at 2.4 GHz after an explicit HAM warm-up.
  - ACT:  true exp via table lookup + fused row-sum (accum_out) on
          half of the [128, 2048] PSUM tiles.
  - DVE:  Schraudolph exp2 bit-trick on the other half: one
          tensor_scalar (psum*A + B -> int32) builds the fp32 bit
          pattern of ~exp(sim); written to SBUF.
  - GPS:  row-sums the bitcast-f32 Schraudolph tiles via
          tensor_scalar(bypass) accum_out.
"""

import os

import numpy as np
import ml_dtypes

import concourse.bass as bass
import concourse.bacc as bacc
import concourse.mybir as mybir
import concourse.tile as tile
from concourse.bass_utils import run_bass_kernel_spmd

N = 8192
D = 256
C = 64
N_CORES = 8
M = N // N_CORES          # 1024 rows per core
P = 128                   # partitions
MT = M // P               # 8 m-tiles per core
QW = 2048                 # PSUM tile width (4 banks)
NQ = N // QW              # 4 q-blocks
CH = QW // 512            # 4 DoubleRow matmuls per tile
NT = NQ * MT              # 32 tiles per core

S = 16.0                  # host prescale of embeddings before fp8 cast
SC_ACT = 2.0 / (S * S)    # exp arg = SC_ACT * psum

LOG2E = 1.4426950408889634
SCH_C = 481196.0          # Schraudolph correction (min mean rel-err)
SCH_A = 2.0 * LOG2E * (1 << 23) / (S * S)
SCH_B = 127.0 * (1 << 23) - SCH_C

# tile t = q*MT + m; True -> ACT (true exp), False -> DVE+GPS (Schraudolph)
# 18 ACT / 14 DVE: ACT's exp+accum is cheaper per tile than the DVE
# convert + GPS fold + DVE reduce chain, so ACT takes the extra tiles.
ASSIGN_ACT = [(t % 2) == 0 or t in (15, 31) for t in range(NT)]
if os.environ.get("K_ALL_ACT"):
    ASSIGN_ACT = [True] * NT
GPS_FOLD = not os.environ.get("K_NO_GPS")   # fold on GPSIMD vs vector
GPS_DMA = not os.environ.get("K_NO_GPS_DMA")  # q2/q3 DMA on SWDGE queue

N_WARM = 0 if os.environ.get("K_NO_WARM") else 10  # junk MMs to warm PE HAM

_F32 = mybir.dt.float32
_BF16 = mybir.dt.bfloat16
_F8 = mybir.dt.float8e4
_I32 = mybir.dt.int32
_F8_NP = ml_dtypes.float8_e4m3fn


def build_nc(enable_asserts: bool = False):
    nc = bacc.Bacc(
        "TRN2",
        target_bir_lowering=False,
        debug=False,
        enable_asserts=enable_asserts,
        num_devices=N_CORES,
    )

    # embT[p, q, c, j, n] = fp8(S * E[q*2048 + c*512 + n, p + 128*j])
    embT = nc.dram_tensor("embT", [P, NQ, CH, 2, 512], _F8, kind="ExternalInput").ap()
    # embTr[p, m, j, mm] = fp8(S * E[core*1024 + m*128 + mm, p + 128*j])
    embTr = nc.dram_tensor("embTr", [P, MT, 2, P], _F8, kind="ExternalInput").ap()
    # parts[:, t] = row-sum over tile t's 2048 columns
    parts_d = nc.dram_tensor("parts", [P, NT], _F32, kind="ExternalOutput").ap()

    with tile.TileContext(nc) as tc:
        with (
            tc.tile_pool(name="big", bufs=1) as big,
            tc.tile_pool(name="conv", bufs=2) as convp,
            tc.tile_pool(name="fold", bufs=2) as foldp,
            tc.tile_pool(name="small", bufs=1) as small,
            tc.tile_pool(name="psum", bufs=2, space=bass.MemorySpace.PSUM) as psum,
        ):
            embT_sb = big.tile([P, NQ, CH, 2, 512], _F8, tag="embT")
            embTr_sb = big.tile([P, MT, 2, P], _F8, tag="embTr")
            parts = small.tile([P, NT], _F32, tag="parts")
            dummy = small.tile([P, 1], _F32, tag="dummy")
            warm_w = small.tile([P, P], _BF16, tag="warmw")
            warm_x = small.tile([P, 512], _BF16, tag="warmx")

            # ---- input DMAs (two queues: sync + gpsimd SWDGE) ----
            nc.sync.dma_start(out=embTr_sb[:], in_=embTr[:])
            nc.sync.dma_start(out=embT_sb[:, 0], in_=embT[:, 0])
            nc.sync.dma_start(out=embT_sb[:, 1], in_=embT[:, 1])
            dma_eng = nc.gpsimd if GPS_DMA else nc.sync
            dma_eng.dma_start(out=embT_sb[:, 2], in_=embT[:, 2])
            dma_eng.dma_start(out=embT_sb[:, 3], in_=embT[:, 3])

            # ---- t=0: hoist ACT exp-table load; HAM warm-up on PE ----
            nc.gpsimd.memset(dummy[:], 0.0)
            nc.scalar.activation(
                out=dummy[:], in_=dummy[:],
                func=mybir.ActivationFunctionType.Exp, bias=0.0, scale=1.0,
            )
            nc.vector.memset(warm_w[:], 0.0)
            nc.vector.memset(warm_x[:], 0.0)
            warm_ps = psum.tile([P, 512], _F32, tag="ps", name="warm_ps")
            for _ in range(N_WARM):
                nc.tensor.matmul(warm_ps[:], lhsT=warm_w[:], rhs=warm_x[:],
                                 start=True, stop=True)

            # ---- main loop: fp8 DoubleRow sim tiles + split exp ----
            for q in range(NQ):
                for m in range(MT):
                    t = q * MT + m
                    ps = psum.tile([P, QW], _F32, tag="ps")
                    for c in range(CH):
                        nc.tensor.matmul(
                            ps[:, c * 512:(c + 1) * 512],
                            lhsT=embTr_sb[:, m],
                            rhs=embT_sb[:, q, c],
                            start=True, stop=True,
                            perf_mode=mybir.MatmulPerfMode.DoubleRow,
                        )
                    if ASSIGN_ACT[t]:
                        nc.scalar.activation(
                            out=ps[:], in_=ps[:],
                            func=mybir.ActivationFunctionType.Exp,
                            bias=0.0, scale=SC_ACT,
                            accum_out=parts[:, t:t + 1],
                        )
                    else:
                        cv = convp.tile([P, QW], _I32, tag="conv")
                        nc.vector.tensor_scalar(
                            out=cv[:], in0=ps[:],
                            scalar1=SCH_A, scalar2=SCH_B,
                            op0=mybir.AluOpType.mult, op1=mybir.AluOpType.add,
                        )
                        # GPSIMD folds the two halves (TENSOR_SCALAR isn't a
                        # legal Pool opcode, TENSOR_TENSOR is); DVE reduces
                        # the folded half.
                        fold = foldp.tile([P, QW // 2], _F32, tag="fold")
                        fold_eng = nc.gpsimd if GPS_FOLD else nc.vector
                        fold_eng.tensor_tensor(
                            fold[:],
                            cv[:, 0:QW // 2].bitcast(_F32),
                            cv[:, QW // 2:QW].bitcast(_F32),
                            op=mybir.AluOpType.add,
                        )
                        nc.vector.tensor_reduce(
                            out=parts[:, t:t + 1], in_=fold[:],
                            axis=mybir.AxisListType.X, op=mybir.AluOpType.add,
                        )

            nc.sync.dma_start(out=parts_d[:], in_=parts[:])

    nc.compile()
    return nc


_NC_CACHE = None


def _get_nc():
    global _NC_CACHE
    if _NC_CACHE is None:
        _NC_CACHE = build_nc()
    return _NC_CACHE


def make_in_maps(embeddings: np.ndarray, labels: np.ndarray):
    emb = np.asarray(embeddings, dtype=np.float32)
    q8 = (S * emb).astype(_F8_NP)                       # [N, D] fp8
    # embT[p, q, c, j, n] = q8[q*2048 + c*512 + n, p + 128*j]
    embT = np.ascontiguousarray(
        q8.reshape(NQ, CH, 512, 2, P).transpose(4, 0, 1, 3, 2)
    )
    in_maps = []
    for core in range(N_CORES):
        r0 = core * M
        # embTr[p, m, j, mm] = q8[r0 + m*128 + mm, p + 128*j]
        embTr = np.ascontiguousarray(
            q8[r0:r0 + M].reshape(MT, P, 2, P).transpose(3, 0, 2, 1)
        )
        in_maps.append({"embT": embT, "embTr": embTr})
    return in_maps


def _schraudolph_np(psum_vals: np.ndarray) -> np.ndarray:
    """Host replica of the device DVE path: fp32(psum*A+B) -> trunc int32
    -> bitcast f32.  psum_vals are device-scale (S^2 * dot)."""
    t = np.float32(psum_vals) * np.float32(SCH_A) + np.float32(SCH_B)
    return np.trunc(t).astype(np.int64).astype(np.int32).view(np.float32)


def finalize(results, embeddings: np.ndarray, labels: np.ndarray) -> np.float32:
    emb = np.asarray(embeddings, dtype=np.float64)
    labels = np.asarray(labels).astype(np.int64)

    # den_full[i] = sum over the 4 q-parts of row i's m-tile
    den_full = np.empty(N, dtype=np.float64)
    for core in range(N_CORES):
        pr = np.asarray(results[core]["parts"], dtype=np.float64)  # [P, NT]
        for m in range(MT):
            rows = core * M + m * P + np.arange(P)
            den_full[rows] = pr[:, [q * MT + m for q in range(NQ)]].sum(axis=1)

    # diagonal correction: subtract what the device added for j == i,
    # which depends on which path (ACT exp vs Schraudolph) owned col i
    q8 = (S * emb.astype(np.float32)).astype(_F8_NP).astype(np.float64)
    diag_psum = (q8 * q8).sum(axis=1)                   # device-scale sim_ii
    rows = np.arange(N)
    m_of = (rows % M) // P
    qp_of = rows // QW % NQ                              # col q-block of diag
    t_of = qp_of * MT + m_of
    is_act = np.array(ASSIGN_ACT)[t_of]
    diag = np.where(
        is_act,
        np.exp(SC_ACT * diag_psum),
        _schraudolph_np(diag_psum.astype(np.float32)).astype(np.float64),
    )
    den = den_full - diag
    logden = np.log(den)

    counts = np.bincount(labels, minlength=C)
    npos = (counts[labels] - 1).astype(np.float64)
    n_pos = npos.sum()

    # positive-pair sim total: sum_{i!=j, lab eq} 2*(e_i.e_j)
    G = np.zeros((C, D), dtype=np.float64)
    np.add.at(G, labels, emb)
    sumsq = (emb * emb).sum(axis=1)
    pos_sim_total = 2.0 * ((G * G).sum() - sumsq.sum())

    numer = (npos * logden).sum() - pos_sim_total
    return np.float32(numer / n_pos)


def _run(inputs, trace: bool = False, **kwargs):
    nc = _get_nc()
    in_maps = make_in_maps(inputs["embeddings"], inputs["epitope_labels"])
    return run_bass_kernel_spmd(nc, in_maps, list(range(N_CORES)), trace=trace, **kwargs)


def kernel(embeddings, epitope_labels) -> np.ndarray:
    res = _run({"embeddings": embeddings, "epitope_labels": epitope_labels})
    return finalize(res.results, embeddings, epitope_labels)


# revision 11
# speedup vs baseline: 1.1965x; 1.1092x over previous
"""Contrastive loss (supervised NT-Xent style) on 8 Trainium2 NeuronCores.

Reference (N=8192, D=256, C=64, T=0.5):
    sim   = (E @ E.T) / T = 2*(e_i . e_j)
    loss  = mean over positive pairs (label match, i != j) of
            (log sum_{j != i} exp(sim_ij) - sim_ij)
(The reference's row-max shift cancels exactly: log-sum-exp + max is
shift-invariant, and |sim| <= 2 so no overflow protection is needed.)

Device work (the N^2 part): den_full_i = sum_j exp(2 e_i.e_j), rows
sharded across 8 cores (1024 rows/core), each against the full 8192
columns. Everything O(N*D) — class sums G_c, ||e_i||^2, the diagonal
correction, bincounts, logs — runs on host in float64.

Per-core engine split (the exp over 8192x8192 is the roofline):
  - PE:   fp8(e4m3) DoubleRow matmuls (K=256 in one pass, 2 fp8
          weights/cell) at 2.4 GHz after an explicit HAM warm-up.
  - ACT:  true exp via table lookup + fused row-sum (accum_out) on
          half of the [128, 2048] PSUM tiles.
  - DVE:  Schraudolph exp2 bit-trick on the other half: one
          tensor_scalar (psum*A + B -> int32) builds the fp32 bit
          pattern of ~exp(sim); written to SBUF.
  - GPS:  row-sums the bitcast-f32 Schraudolph tiles via
          tensor_scalar(bypass) accum_out.
"""

import os

import numpy as np
import ml_dtypes

import concourse.bass as bass
import concourse.bacc as bacc
import concourse.mybir as mybir
import concourse.tile as tile
from concourse.bass_utils import run_bass_kernel_spmd

N = 8192
D = 256
C = 64
N_CORES = 8
M = N // N_CORES          # 1024 rows per core
P = 128                   # partitions
MT = M // P               # 8 m-tiles per core
QW = 2048                 # PSUM tile width (4 banks)
NQ = N // QW              # 4 q-blocks
CH = QW // 512            # 4 DoubleRow matmuls per tile
NT = NQ * MT              # 32 tiles per core

S = 16.0                  # host prescale of embeddings before fp8 cast
SC_ACT = 2.0 / (S * S)    # exp arg = SC_ACT * psum

LOG2E = 1.4426950408889634
SCH_C = 481196.0          # Schraudolph correction (min mean rel-err)
SCH_A = 2.0 * LOG2E * (1 << 23) / (S * S)
SCH_B = 127.0 * (1 << 23) - SCH_C

# tile t = q*MT + m; True -> ACT (true exp), False -> DVE+GPS (Schraudolph)
# 18 ACT / 14 DVE: ACT's exp+accum is cheaper per tile than the DVE
# convert + GPS fold + DVE reduce chain, so ACT takes the extra tiles.
ASSIGN_ACT = [(t % 2) == 0 or t in (15, 31) for t in range(NT)]
if os.environ.get("K_ALL_ACT"):
    ASSIGN_ACT = [True] * NT
GPS_FOLD = not os.environ.get("K_NO_GPS")   # fold on GPSIMD vs vector
# q2/q3 DMA on the GPSIMD SWDGE queue hangs the device when GPSIMD also
# runs the fold TENSOR_TENSORs (NRT_EXEC_UNIT_UNRECOVERABLE) — keep the
# streaming DMAs on the sync HWDGE queue.
GPS_DMA = bool(os.environ.get("K_GPS_DMA"))

N_WARM = 0 if os.environ.get("K_NO_WARM") else 10  # junk MMs to warm PE HAM

_F32 = mybir.dt.float32
_BF16 = mybir.dt.bfloat16
_F8 = mybir.dt.float8e4
_I32 = mybir.dt.int32
_F8_NP = ml_dtypes.float8_e4m3fn


def build_nc(enable_asserts: bool = False):
    nc = bacc.Bacc(
        "TRN2",
        target_bir_lowering=False,
        debug=False,
        enable_asserts=enable_asserts,
        num_devices=N_CORES,
    )

    # embT[p, q, c, j, n] = fp8(S * E[q*2048 + c*512 + n, p + 128*j])
    embT = nc.dram_tensor("embT", [P, NQ, CH, 2, 512], _F8, kind="ExternalInput").ap()
    # embTr[p, m, j, mm] = fp8(S * E[core*1024 + m*128 + mm, p + 128*j])
    embTr = nc.dram_tensor("embTr", [P, MT, 2, P], _F8, kind="ExternalInput").ap()
    # parts[:, t] = row-sum over tile t's 2048 columns
    parts_d = nc.dram_tensor("parts", [P, NT], _F32, kind="ExternalOutput").ap()

    with tile.TileContext(nc) as tc:
        with (
            tc.tile_pool(name="big", bufs=1) as big,
            tc.tile_pool(name="conv", bufs=2) as convp,
            tc.tile_pool(name="fold", bufs=2) as foldp,
            tc.tile_pool(name="small", bufs=1) as small,
            tc.tile_pool(name="psum", bufs=2, space=bass.MemorySpace.PSUM) as psum,
        ):
            embT_sb = big.tile([P, NQ, CH, 2, 512], _F8, tag="embT")
            embTr_sb = big.tile([P, MT, 2, P], _F8, tag="embTr")
            parts = small.tile([P, NT], _F32, tag="parts")
            dummy = small.tile([P, 1], _F32, tag="dummy")
            warm_w = small.tile([P, P], _BF16, tag="warmw")
            warm_x = small.tile([P, 512], _BF16, tag="warmx")

            # ---- input DMAs (two queues: sync + gpsimd SWDGE) ----
            nc.sync.dma_start(out=embTr_sb[:], in_=embTr[:])
            nc.sync.dma_start(out=embT_sb[:, 0], in_=embT[:, 0])
            nc.sync.dma_start(out=embT_sb[:, 1], in_=embT[:, 1])
            dma_eng = nc.gpsimd if GPS_DMA else nc.sync
            dma_eng.dma_start(out=embT_sb[:, 2], in_=embT[:, 2])
            dma_eng.dma_start(out=embT_sb[:, 3], in_=embT[:, 3])

            # ---- t=0: hoist ACT exp-table load; HAM warm-up on PE ----
            nc.gpsimd.memset(dummy[:], 0.0)
            nc.scalar.activation(
                out=dummy[:], in_=dummy[:],
                func=mybir.ActivationFunctionType.Exp, bias=0.0, scale=1.0,
            )
            nc.vector.memset(warm_w[:], 0.0)
            nc.vector.memset(warm_x[:], 0.0)
            warm_ps = psum.tile([P, 512], _F32, tag="ps", name="warm_ps")
            for _ in range(N_WARM):
                nc.tensor.matmul(warm_ps[:], lhsT=warm_w[:], rhs=warm_x[:],
                                 start=True, stop=True)

            # ---- main loop: fp8 DoubleRow sim tiles + split exp ----
            for q in range(NQ):
                for m in range(MT):
                    t = q * MT + m
                    ps = psum.tile([P, QW], _F32, tag="ps")
                    for c in range(CH):
                        nc.tensor.matmul(
                            ps[:, c * 512:(c + 1) * 512],
                            lhsT=embTr_sb[:, m],
                            rhs=embT_sb[:, q, c],
                            start=True, stop=True,
                            perf_mode=mybir.MatmulPerfMode.DoubleRow,
                        )
                    if ASSIGN_ACT[t]:
                        nc.scalar.activation(
                            out=ps[:], in_=ps[:],
                            func=mybir.ActivationFunctionType.Exp,
                            bias=0.0, scale=SC_ACT,
                            accum_out=parts[:, t:t + 1],
                        )
                    else:
                        cv = convp.tile([P, QW], _I32, tag="conv")
                        nc.vector.tensor_scalar(
                            out=cv[:], in0=ps[:],
                            scalar1=SCH_A, scalar2=SCH_B,
                            op0=mybir.AluOpType.mult, op1=mybir.AluOpType.add,
                        )
                        # GPSIMD folds the two halves (TENSOR_SCALAR isn't a
                        # legal Pool opcode, TENSOR_TENSOR is); DVE reduces
                        # the folded half.
                        fold = foldp.tile([P, QW // 2], _F32, tag="fold")
                        fold_eng = nc.gpsimd if GPS_FOLD else nc.vector
                        fold_eng.tensor_tensor(
                            fold[:],
                            cv[:, 0:QW // 2].bitcast(_F32),
                            cv[:, QW // 2:QW].bitcast(_F32),
                            op=mybir.AluOpType.add,
                        )
                        nc.vector.tensor_reduce(
                            out=parts[:, t:t + 1], in_=fold[:],
                            axis=mybir.AxisListType.X, op=mybir.AluOpType.add,
                        )

            nc.sync.dma_start(out=parts_d[:], in_=parts[:])

    nc.compile()
    return nc


_NC_CACHE = None


def _get_nc():
    global _NC_CACHE
    if _NC_CACHE is None:
        _NC_CACHE = build_nc()
    return _NC_CACHE


def make_in_maps(embeddings: np.ndarray, labels: np.ndarray):
    emb = np.asarray(embeddings, dtype=np.float32)
    q8 = (S * emb).astype(_F8_NP)                       # [N, D] fp8
    # embT[p, q, c, j, n] = q8[q*2048 + c*512 + n, p + 128*j]
    embT = np.ascontiguousarray(
        q8.reshape(NQ, CH, 512, 2, P).transpose(4, 0, 1, 3, 2)
    )
    in_maps = []
    for core in range(N_CORES):
        r0 = core * M
        # embTr[p, m, j, mm] = q8[r0 + m*128 + mm, p + 128*j]
        embTr = np.ascontiguousarray(
            q8[r0:r0 + M].reshape(MT, P, 2, P).transpose(3, 0, 2, 1)
        )
        in_maps.append({"embT": embT, "embTr": embTr})
    return in_maps


def _schraudolph_np(psum_vals: np.ndarray) -> np.ndarray:
    """Host replica of the device DVE path: fp32(psum*A+B) -> trunc int32
    -> bitcast f32.  psum_vals are device-scale (S^2 * dot)."""
    t = np.float32(psum_vals) * np.float32(SCH_A) + np.float32(SCH_B)
    return np.trunc(t).astype(np.int64).astype(np.int32).view(np.float32)


def finalize(results, embeddings: np.ndarray, labels: np.ndarray) -> np.float32:
    emb = np.asarray(embeddings, dtype=np.float64)
    labels = np.asarray(labels).astype(np.int64)

    # den_full[i] = sum over the 4 q-parts of row i's m-tile
    den_full = np.empty(N, dtype=np.float64)
    for core in range(N_CORES):
        pr = np.asarray(results[core]["parts"], dtype=np.float64)  # [P, NT]
        for m in range(MT):
            rows = core * M + m * P + np.arange(P)
            den_full[rows] = pr[:, [q * MT + m for q in range(NQ)]].sum(axis=1)

    # diagonal correction: subtract what the device added for j == i,
    # which depends on which path (ACT exp vs Schraudolph) owned col i
    q8 = (S * emb.astype(np.float32)).astype(_F8_NP).astype(np.float64)
    diag_psum = (q8 * q8).sum(axis=1)                   # device-scale sim_ii
    rows = np.arange(N)
    m_of = (rows % M) // P
    qp_of = rows // QW % NQ                              # col q-block of diag
    t_of = qp_of * MT + m_of
    is_act = np.array(ASSIGN_ACT)[t_of]
    diag = np.where(
        is_act,
        np.exp(SC_ACT * diag_psum),
        _schraudolph_np(diag_psum.astype(np.float32)).astype(np.float64),
    )
    den = den_full - diag
    logden = np.log(den)

    counts = np.bincount(labels, minlength=C)
    npos = (counts[labels] - 1).astype(np.float64)
    n_pos = npos.sum()

    # positive-pair sim total: sum_{i!=j, lab eq} 2*(e_i.e_j)
    G = np.zeros((C, D), dtype=np.float64)
    np.add.at(G, labels, emb)
    sumsq = (emb * emb).sum(axis=1)
    pos_sim_total = 2.0 * ((G * G).sum() - sumsq.sum())

    numer = (npos * logden).sum() - pos_sim_total
    return np.float32(numer / n_pos)


def _run(inputs, trace: bool = False, **kwargs):
    nc = _get_nc()
    in_maps = make_in_maps(inputs["embeddings"], inputs["epitope_labels"])
    return run_bass_kernel_spmd(nc, in_maps, list(range(N_CORES)), trace=trace, **kwargs)


def kernel(embeddings, epitope_labels) -> np.ndarray:
    res = _run({"embeddings": embeddings, "epitope_labels": epitope_labels})
    return finalize(res.results, embeddings, epitope_labels)


# revision 14
# speedup vs baseline: 1.3503x; 1.1286x over previous
"""Contrastive loss (supervised NT-Xent style) on 8 Trainium2 NeuronCores.

Reference (N=8192, D=256, C=64, T=0.5):
    sim   = (E @ E.T) / T = 2*(e_i . e_j)
    loss  = mean over positive pairs (label match, i != j) of
            (log sum_{j != i} exp(sim_ij) - sim_ij)
(The reference's row-max shift cancels exactly: log-sum-exp + max is
shift-invariant, and |sim| <= 2 so no overflow protection is needed.)

Device work (the N^2 part): den_full_i = sum_j exp(2 e_i.e_j), rows
sharded across 8 cores (1024 rows/core), each against the full 8192
columns. Everything O(N*D) — class sums G_c, ||e_i||^2, the diagonal
correction, bincounts, logs — runs on host in float64.

Per-core engine split (the exp over 8192x8192 is the roofline):
  - PE:   fp8(e4m3) DoubleRow matmuls (K=256 in one pass, 2 fp8
          weights/cell) at 2.4 GHz after an explicit HAM warm-up.
  - ACT:  true exp via table lookup + fused row-sum (accum_out) on
          half of the [128, 2048] PSUM tiles.
  - DVE:  Schraudolph exp2 bit-trick on the other half: one
          tensor_scalar (psum*A + B -> int32) builds the fp32 bit
          pattern of ~exp(sim); written to SBUF.
  - GPS:  row-sums the bitcast-f32 Schraudolph tiles via
          tensor_scalar(bypass) accum_out.
"""

import os

import numpy as np
import ml_dtypes

import concourse.bass as bass
import concourse.bacc as bacc
import concourse.mybir as mybir
import concourse.tile as tile
from concourse.bass_utils import run_bass_kernel_spmd

N = 8192
D = 256
C = 64
N_CORES = 8
M = N // N_CORES          # 1024 rows per core
P = 128                   # partitions
MT = M // P               # 8 m-tiles per core
QW = 2048                 # PSUM tile width (4 banks)
NQ = N // QW              # 4 q-blocks
CH = QW // 512            # 4 DoubleRow matmuls per tile
NT = NQ * MT              # 32 tiles per core

S = 16.0                  # host prescale of embeddings before fp8 cast
SC_ACT = 2.0 / (S * S)    # exp arg = SC_ACT * psum

LOG2E = 1.4426950408889634
SCH_C = 481196.0          # Schraudolph correction (min mean rel-err)
SCH_A = 2.0 * LOG2E * (1 << 23) / (S * S)
SCH_B = 127.0 * (1 << 23) - SCH_C

# tile t = q*MT + m; True -> ACT (true exp), False -> DVE+GPS (Schraudolph)
# 18 ACT / 14 DVE: ACT's exp+accum is cheaper per tile than the DVE
# convert + GPS fold + DVE reduce chain, so ACT takes the extra tiles.
ASSIGN_ACT = [(t % 2) == 0 or (t % 16) > 11 for t in range(NT)]
if os.environ.get("K_ALL_ACT"):
    ASSIGN_ACT = [True] * NT
GPS_FOLD = not os.environ.get("K_NO_GPS")   # fold on GPSIMD vs vector
# q2/q3 DMA on the GPSIMD SWDGE queue hangs the device when GPSIMD also
# runs the fold TENSOR_TENSORs (NRT_EXEC_UNIT_UNRECOVERABLE) — keep the
# streaming DMAs on the sync HWDGE queue.
GPS_DMA = bool(os.environ.get("K_GPS_DMA"))

N_WARM = 0 if os.environ.get("K_NO_WARM") else 10  # junk MMs to warm PE HAM

_F32 = mybir.dt.float32
_BF16 = mybir.dt.bfloat16
_F8 = mybir.dt.float8e4
_I32 = mybir.dt.int32
_F8_NP = ml_dtypes.float8_e4m3fn


def build_nc(enable_asserts: bool = False):
    nc = bacc.Bacc(
        "TRN2",
        target_bir_lowering=False,
        debug=False,
        enable_asserts=enable_asserts,
        num_devices=N_CORES,
    )

    # embT[p, q, c, j, n] = fp8(S * E[q*2048 + c*512 + n, p + 128*j])
    embT = nc.dram_tensor("embT", [P, NQ, CH, 2, 512], _F8, kind="ExternalInput").ap()
    # embTr[p, m, j, mm] = fp8(S * E[core*1024 + m*128 + mm, p + 128*j])
    embTr = nc.dram_tensor("embTr", [P, MT, 2, P], _F8, kind="ExternalInput").ap()
    # parts[:, t] = row-sum over tile t's 2048 columns
    parts_d = nc.dram_tensor("parts", [P, NT], _F32, kind="ExternalOutput").ap()

    with tile.TileContext(nc) as tc:
        with (
            tc.tile_pool(name="big", bufs=1) as big,
            tc.tile_pool(name="conv", bufs=2) as convp,
            tc.tile_pool(name="fold", bufs=3) as foldp,
            tc.tile_pool(name="small", bufs=1) as small,
            tc.tile_pool(name="psum", bufs=2, space=bass.MemorySpace.PSUM) as psum,
        ):
            embT_sb = big.tile([P, NQ, CH, 2, 512], _F8, tag="embT")
            embTr_sb = big.tile([P, MT, 2, P], _F8, tag="embTr")
            parts = small.tile([P, NT], _F32, tag="parts")
            dummy = small.tile([P, 1], _F32, tag="dummy")
            warm_w = small.tile([P, P], _BF16, tag="warmw")
            warm_x = small.tile([P, 512], _BF16, tag="warmx")

            # ---- input DMAs (two queues: sync + gpsimd SWDGE) ----
            nc.sync.dma_start(out=embTr_sb[:], in_=embTr[:])
            nc.sync.dma_start(out=embT_sb[:, 0], in_=embT[:, 0])
            nc.sync.dma_start(out=embT_sb[:, 1], in_=embT[:, 1])
            dma_eng = nc.gpsimd if GPS_DMA else nc.sync
            dma_eng.dma_start(out=embT_sb[:, 2], in_=embT[:, 2])
            dma_eng.dma_start(out=embT_sb[:, 3], in_=embT[:, 3])

            # ---- t=0: hoist ACT exp-table load; HAM warm-up on PE ----
            nc.gpsimd.memset(dummy[:], 0.0)
            nc.scalar.activation(
                out=dummy[:], in_=dummy[:],
                func=mybir.ActivationFunctionType.Exp, bias=0.0, scale=1.0,
            )
            nc.vector.memset(warm_w[:], 0.0)
            nc.vector.memset(warm_x[:], 0.0)
            warm_ps = psum.tile([P, 512], _F32, tag="ps", name="warm_ps")
            for _ in range(N_WARM):
                nc.tensor.matmul(warm_ps[:], lhsT=warm_w[:], rhs=warm_x[:],
                                 start=True, stop=True)

            # ---- main loop: fp8 DoubleRow sim tiles + split exp ----
            # The DVE engine queue is strict FIFO: a tensor_reduce that
            # waits on its GPSIMD fold would block later tensor_scalars.
            # Software-pipeline: issue each DVE tile's reduce only after
            # the NEXT DVE tile's convert, so the fold runs in the gap.
            pending = None      # (fold_tile, t) awaiting its reduce
            for q in range(NQ):
                for m in range(MT):
                    t = q * MT + m
                    ps = psum.tile([P, QW], _F32, tag="ps")
                    for c in range(CH):
                        nc.tensor.matmul(
                            ps[:, c * 512:(c + 1) * 512],
                            lhsT=embTr_sb[:, m],
                            rhs=embT_sb[:, q, c],
                            start=True, stop=True,
                            perf_mode=mybir.MatmulPerfMode.DoubleRow,
                        )
                    if ASSIGN_ACT[t]:
                        nc.scalar.activation(
                            out=ps[:], in_=ps[:],
                            func=mybir.ActivationFunctionType.Exp,
                            bias=0.0, scale=SC_ACT,
                            accum_out=parts[:, t:t + 1],
                        )
                    else:
                        cv = convp.tile([P, QW], _I32, tag="conv")
                        nc.vector.tensor_scalar(
                            out=cv[:], in0=ps[:],
                            scalar1=SCH_A, scalar2=SCH_B,
                            op0=mybir.AluOpType.mult, op1=mybir.AluOpType.add,
                        )
                        # GPSIMD folds the two halves (TENSOR_SCALAR isn't a
                        # legal Pool opcode, TENSOR_TENSOR is); DVE reduces
                        # the folded half one tile later.
                        fold = foldp.tile([P, QW // 2], _F32, tag="fold")
                        fold_eng = nc.gpsimd if GPS_FOLD else nc.vector
                        fold_eng.tensor_tensor(
                            fold[:],
                            cv[:, 0:QW // 2].bitcast(_F32),
                            cv[:, QW // 2:QW].bitcast(_F32),
                            op=mybir.AluOpType.add,
                        )
                        if pending is not None:
                            pf, pt = pending
                            nc.vector.tensor_reduce(
                                out=parts[:, pt:pt + 1], in_=pf[:],
                                axis=mybir.AxisListType.X, op=mybir.AluOpType.add,
                            )
                        pending = (fold, t)
            if pending is not None:
                pf, pt = pending
                nc.vector.tensor_reduce(
                    out=parts[:, pt:pt + 1], in_=pf[:],
                    axis=mybir.AxisListType.X, op=mybir.AluOpType.add,
                )

            nc.sync.dma_start(out=parts_d[:], in_=parts[:])

    nc.compile()
    return nc


_NC_CACHE = None


def _get_nc():
    global _NC_CACHE
    if _NC_CACHE is None:
        _NC_CACHE = build_nc()
    return _NC_CACHE


def make_in_maps(embeddings: np.ndarray, labels: np.ndarray):
    emb = np.asarray(embeddings, dtype=np.float32)
    q8 = (S * emb).astype(_F8_NP)                       # [N, D] fp8
    # embT[p, q, c, j, n] = q8[q*2048 + c*512 + n, p + 128*j]
    embT = np.ascontiguousarray(
        q8.reshape(NQ, CH, 512, 2, P).transpose(4, 0, 1, 3, 2)
    )
    in_maps = []
    for core in range(N_CORES):
        r0 = core * M
        # embTr[p, m, j, mm] = q8[r0 + m*128 + mm, p + 128*j]
        embTr = np.ascontiguousarray(
            q8[r0:r0 + M].reshape(MT, P, 2, P).transpose(3, 0, 2, 1)
        )
        in_maps.append({"embT": embT, "embTr": embTr})
    return in_maps


def _schraudolph_np(psum_vals: np.ndarray) -> np.ndarray:
    """Host replica of the device DVE path: fp32(psum*A+B) -> trunc int32
    -> bitcast f32.  psum_vals are device-scale (S^2 * dot)."""
    t = np.float32(psum_vals) * np.float32(SCH_A) + np.float32(SCH_B)
    return np.trunc(t).astype(np.int64).astype(np.int32).view(np.float32)


def finalize(results, embeddings: np.ndarray, labels: np.ndarray) -> np.float32:
    emb = np.asarray(embeddings, dtype=np.float64)
    labels = np.asarray(labels).astype(np.int64)

    # den_full[i] = sum over the 4 q-parts of row i's m-tile
    den_full = np.empty(N, dtype=np.float64)
    for core in range(N_CORES):
        pr = np.asarray(results[core]["parts"], dtype=np.float64)  # [P, NT]
        for m in range(MT):
            rows = core * M + m * P + np.arange(P)
            den_full[rows] = pr[:, [q * MT + m for q in range(NQ)]].sum(axis=1)

    # diagonal correction: subtract what the device added for j == i,
    # which depends on which path (ACT exp vs Schraudolph) owned col i
    q8 = (S * emb.astype(np.float32)).astype(_F8_NP).astype(np.float64)
    diag_psum = (q8 * q8).sum(axis=1)                   # device-scale sim_ii
    rows = np.arange(N)
    m_of = (rows % M) // P
    qp_of = rows // QW % NQ                              # col q-block of diag
    t_of = qp_of * MT + m_of
    is_act = np.array(ASSIGN_ACT)[t_of]
    diag = np.where(
        is_act,
        np.exp(SC_ACT * diag_psum),
        _schraudolph_np(diag_psum.astype(np.float32)).astype(np.float64),
    )
    den = den_full - diag
    logden = np.log(den)

    counts = np.bincount(labels, minlength=C)
    npos = (counts[labels] - 1).astype(np.float64)
    n_pos = npos.sum()

    # positive-pair sim total: sum_{i!=j, lab eq} 2*(e_i.e_j)
    G = np.zeros((C, D), dtype=np.float64)
    np.add.at(G, labels, emb)
    sumsq = (emb * emb).sum(axis=1)
    pos_sim_total = 2.0 * ((G * G).sum() - sumsq.sum())

    numer = (npos * logden).sum() - pos_sim_total
    return np.float32(numer / n_pos)


def _run(inputs, trace: bool = False, **kwargs):
    nc = _get_nc()
    in_maps = make_in_maps(inputs["embeddings"], inputs["epitope_labels"])
    return run_bass_kernel_spmd(nc, in_maps, list(range(N_CORES)), trace=trace, **kwargs)


def kernel(embeddings, epitope_labels) -> np.ndarray:
    res = _run({"embeddings": embeddings, "epitope_labels": epitope_labels})
    return finalize(res.results, embeddings, epitope_labels)
